# revision 8
# baseline (speedup 1.0000x reference)
"""Expert-parallel MoE (DBRX-style SwiGLU FFN) on 8 TRN2 NeuronCores.

Strategy: one expert per core. Routing (gather tokens per expert, combine
weights) happens on the host; each core runs the SwiGLU FFN for its expert
over its gathered tokens in "transposed activation" form:

    up^T   = w_up^T  @ x^T    (K = H, accumulate over 8 K-subtiles)
    gate^T = w_gate^T @ x^T
    h^T    = silu(up^T) * gate^T          (bf16)
    y^T    = w_down^T @ h^T   (K = F, accumulate over 16 K-subtiles)

All matmuls are bf16 with f32 PSUM accumulation. The host applies the
per-(token, expert) combine weight during the scatter-add.

Head-latency engineering (measured on HW):
 - DMA descriptor issue costs ~610ns per instruction on an engine queue, so
   input DMAs are split across BOTH HWDGE engines (sync + scalar) and
   batched (weight tiles in pairs, x^T as one 1MB transfer) so the first
   matmul's inputs land as early as possible (~9us, right after the ~7us
   framework preamble).
 - A short 5-matmul junk warmup keeps the PE busy from the preamble end
   until the first weights land, which also starts the HAM un-throttle
   window (PE runs at 1.2 GHz until ~3.4us of sustained activity).
 - y^T is written as bf16 (quantization ~0.1% rms, well inside the 2e-2
   budget) halving output DMA, and the final output chunk is 128 cols so
   the last matmul->copy->DMA chain is short.
"""

import numpy as np
import ml_dtypes

import concourse.bacc as bacc
import concourse.mybir as mybir
import concourse.tile as tile
from concourse import bass_utils

HIDDEN = 1024
FFN = 2048
N_EXPERTS = 8
P = 128
KO_H = HIDDEN // P   # 8   K-subtiles for up/gate
KO_F = FFN // P      # 16  K-subtiles for down
FC_N = FFN // P      # 16  F-chunks (output partition tiles of stage A)
HC_N = HIDDEN // P   # 8   H-chunks (output partition tiles of stage B)
FCP_N = FC_N // 2    # 8   F-chunk pairs (one weight DMA each)
HCP_N = HC_N // 2    # 4   H-chunk pairs

BF16 = ml_dtypes.bfloat16

_compiled = {}  # cap -> compiled Bacc module


def _build(cap: int):
    f32 = mybir.dt.float32
    bf16 = mybir.dt.bfloat16
    tchunks = [(t0, min(512, cap - t0)) for t0 in range(0, cap, 512)]

    nc = bacc.Bacc("TRN2", debug=False, enable_asserts=False,
                   num_devices=N_EXPERTS)
    xT_d = nc.dram_tensor("xT", [P, KO_H, cap], bf16, kind="ExternalInput")
    wu_d = nc.dram_tensor("wu", [FCP_N, P, 2, KO_H, P], bf16,
                          kind="ExternalInput")
    wg_d = nc.dram_tensor("wg", [FCP_N, P, 2, KO_H, P], bf16,
                          kind="ExternalInput")
    wd_d = nc.dram_tensor("wd", [HCP_N, P, 2, KO_F, P], bf16,
                          kind="ExternalInput")
    yT_d = nc.dram_tensor("yT", [HC_N, P, cap], bf16, kind="ExternalOutput")

    with tile.TileContext(nc) as tc:
        with (
            tc.tile_pool(name="persist", bufs=1) as persist,
            tc.tile_pool(name="wpool", bufs=3) as wpool,
            tc.tile_pool(name="spool", bufs=4) as spool,
            tc.tile_pool(name="psum", bufs=2, space="PSUM") as psum,
        ):
            # Junk warmup: occupy the PE from preamble-end until the first
            # weights land (~2us) and start the HAM un-throttle window.
            # Alternate PSUM banks so consecutive junk matmuls don't
            # serialize on the bank WAW hazard.
            warm = persist.tile([P, 512], bf16, tag="warm")
            nc.gpsimd.memset(warm[:], 0)
            pw = [psum.tile([P, 512], f32, tag="pwarm", name="pwarm")
                  for _ in range(2)]
            # 9 x 427ns (cold) bridges the PE from preamble-end (~7.9us)
            # to first-weights-landed (~11.7us) with no idle gap, so the
            # HAM un-throttle fires before the first real matmul.
            for i in range(9):
                nc.tensor.matmul(pw[i % 2], warm[:, :P], warm, start=True,
                                 stop=True)

            xT = persist.tile([P, KO_H, cap], bf16, tag="xT")
            ht = persist.tile([P, KO_F, cap], bf16, tag="ht")

            # Stage A: h^T[fc] = silu(up^T) * gate^T, per 128-wide F-chunk.
            # All input DMAs go on ONE queue in exact demand order (two
            # concurrent HWDGE queues starve each other on the shared DMA
            # engines), and x^T is quartered so the first accumulation
            # chain starts as soon as its first slices land.
            for fcp in range(FCP_N):
                wu_t = wpool.tile([P, 2, KO_H, P], bf16, tag="wu")
                wg_t = wpool.tile([P, 2, KO_H, P], bf16, tag="wg")
                if fcp == 0:
                    nc.sync.dma_start(wu_t[:, 0], wu_d.ap()[0][:, 0])
                    for q in range(4):
                        nc.sync.dma_start(xT[:, 2 * q:2 * q + 2],
                                          xT_d.ap()[:, 2 * q:2 * q + 2])
                    nc.sync.dma_start(wg_t[:, 0], wg_d.ap()[0][:, 0])
                    nc.sync.dma_start(wu_t[:, 1], wu_d.ap()[0][:, 1])
                    nc.sync.dma_start(wg_t[:, 1], wg_d.ap()[0][:, 1])
                else:
                    nc.sync.dma_start(wu_t[:], wu_d.ap()[fcp])
                    nc.sync.dma_start(wg_t[:], wg_d.ap()[fcp])
                for half in range(2):
                    fc = 2 * fcp + half
                    for t0, nt in tchunks:
                        pu = psum.tile([P, 512], f32, tag="pu",
                                       name="pu")[:, :nt]
                        pg = psum.tile([P, 512], f32, tag="pg",
                                       name="pg")[:, :nt]
                        for ko in range(KO_H):
                            nc.tensor.matmul(pu, wu_t[:, half, ko],
                                             xT[:, ko, t0:t0 + nt],
                                             start=(ko == 0),
                                             stop=(ko == KO_H - 1))
                        for ko in range(KO_H):
                            nc.tensor.matmul(pg, wg_t[:, half, ko],
                                             xT[:, ko, t0:t0 + nt],
                                             start=(ko == 0),
                                             stop=(ko == KO_H - 1))
                        su = spool.tile([P, 512], f32, tag="silu",
                                        name="su")[:, :nt]
                        nc.scalar.activation(
                            su, pu, mybir.ActivationFunctionType.Sigmoid)
                        nc.vector.tensor_mul(su, su, pu)
                        nc.vector.tensor_mul(ht[:, fc, t0:t0 + nt], su, pg)

            # Stage B: y^T[hc] = w_down^T @ h^T, per 128-wide H-chunk
            for hcp in range(HCP_N):
                wd_t = wpool.tile([P, 2, KO_F, P], bf16, tag="wd")
                nc.sync.dma_start(wd_t[:], wd_d.ap()[hcp])
                for half in range(2):
                    hc = 2 * hcp + half
                    # shrink the final chunks so the PSUM-drain + DMA-out
                    # chain after the very last matmul is short
                    chunks = tchunks
                    if hc == HC_N - 1 and tchunks[-1][1] > 384:
                        t0l, ntl = tchunks[-1]
                        chunks = tchunks[:-1] + [(t0l, 256),
                                                 (t0l + 256, (ntl - 256) // 2),
                                                 (t0l + 256 + (ntl - 256) // 2,
                                                  ntl - 256 - (ntl - 256) // 2)]
                    for t0, nt in chunks:
                        py = psum.tile([P, 512], f32, tag="py",
                                       name="py")[:, :nt]
                        for ko in range(KO_F):
                            nc.tensor.matmul(py, wd_t[:, half, ko],
                                             ht[:, ko, t0:t0 + nt],
                                             start=(ko == 0),
                                             stop=(ko == KO_F - 1))
                        yo = spool.tile([P, 512], bf16, tag="yo",
                                        name="yo")[:, :nt]
                        nc.vector.tensor_copy(yo, py)
                        nc.sync.dma_start(yT_d.ap()[hc][:, t0:t0 + nt], yo)

    nc.compile()
    return nc


def kernel(x, weights, top_weights, top_experts, w_up, w_gate, w_down):
    x = np.asarray(x, dtype=np.float32)
    tw = np.asarray(top_weights, dtype=np.float32)
    te = np.asarray(top_experts).astype(np.int64)
    w_up = np.asarray(w_up, dtype=np.float32)
    w_gate = np.asarray(w_gate, dtype=np.float32)
    w_down = np.asarray(w_down, dtype=np.float32)

    B, S, H = x.shape
    T = B * S
    xf = x.reshape(T, H)

    # --- host routing ---
    idxs, combine = [], []
    for e in range(N_EXPERTS):
        sel = te == e                       # [T, K]
        mask = sel.any(axis=1)
        idx = np.nonzero(mask)[0]
        w_tok = (tw * sel).sum(axis=1)      # [T]
        idxs.append(idx)
        combine.append(w_tok[idx].astype(np.float32))
    max_n = max(len(i) for i in idxs)
    cap = max(-(-max_n // 8) * 8, P)  # PSUM free dim <= 512 f32 per bank

    # --- per-core inputs ---
    in_maps = []
    for e in range(N_EXPERTS):
        idx = idxs[e]
        xg = np.zeros((cap, H), np.float32)
        xg[: len(idx)] = xf[idx]
        # xT[p, ko, t] = xg[t, ko*128+p]
        xT = np.ascontiguousarray(
            xg.T.reshape(KO_H, P, cap).transpose(1, 0, 2)).astype(BF16)
        # [fcp, p, j, ko, q] = w_up[ko*128+p, (2*fcp+j)*128+q]
        wu = np.ascontiguousarray(
            w_up[e].reshape(KO_H, P, FCP_N, 2, P)
            .transpose(2, 1, 3, 0, 4)).astype(BF16)
        wg = np.ascontiguousarray(
            w_gate[e].reshape(KO_H, P, FCP_N, 2, P)
            .transpose(2, 1, 3, 0, 4)).astype(BF16)
        wd = np.ascontiguousarray(
            w_down[e].reshape(KO_F, P, HCP_N, 2, P)
            .transpose(2, 1, 3, 0, 4)).astype(BF16)
        in_maps.append({"xT": xT, "wu": wu, "wg": wg, "wd": wd})

    # --- compile (cached) + run ---
    if cap not in _compiled:
        _compiled[cap] = _build(cap)
    nc = _compiled[cap]
    res = bass_utils.run_bass_kernel_spmd(
        nc, in_maps, core_ids=list(range(N_EXPERTS)))

    # --- combine on host ---
    out = np.zeros((T, H), np.float32)
    for e in range(N_EXPERTS):
        idx = idxs[e]
        yT = res.results[e]["yT"].astype(np.float32).reshape(H, cap)
        out[idx] += yT[:, : len(idx)].T * combine[e][:, None]
    return out.reshape(B, S, H)


# revision 9
# speedup vs baseline: 1.0133x; 1.0133x over previous
"""Expert-parallel MoE (DBRX-style SwiGLU FFN) on 8 TRN2 NeuronCores.

Strategy: one expert per core. Routing (gather tokens per expert, combine
weights) happens on the host; each core runs the SwiGLU FFN for its expert
over its gathered tokens in "transposed activation" form:

    up^T   = w_up^T  @ x^T    (K = H, accumulate over 8 K-subtiles)
    gate^T = w_gate^T @ x^T
    h^T    = silu(up^T) * gate^T          (bf16)
    y^T    = w_down^T @ h^T   (K = F, accumulate over 16 K-subtiles)

All matmuls are bf16 with f32 PSUM accumulation. The host applies the
per-(token, expert) combine weight during the scatter-add.

Head-latency engineering (measured on HW):
 - DMA descriptor issue costs ~610ns per instruction on an engine queue, so
   input DMAs are split across BOTH HWDGE engines (sync + scalar) and
   batched (weight tiles in pairs, x^T as one 1MB transfer) so the first
   matmul's inputs land as early as possible (~9us, right after the ~7us
   framework preamble).
 - A short 5-matmul junk warmup keeps the PE busy from the preamble end
   until the first weights land, which also starts the HAM un-throttle
   window (PE runs at 1.2 GHz until ~3.4us of sustained activity).
 - y^T is written as bf16 (quantization ~0.1% rms, well inside the 2e-2
   budget) halving output DMA, and the final output chunk is 128 cols so
   the last matmul->copy->DMA chain is short.
"""

import numpy as np
import ml_dtypes

import concourse.bacc as bacc
import concourse.mybir as mybir
import concourse.tile as tile
from concourse import bass_utils

HIDDEN = 1024
FFN = 2048
N_EXPERTS = 8
P = 128
KO_H = HIDDEN // P   # 8   K-subtiles for up/gate
KO_F = FFN // P      # 16  K-subtiles for down
FC_N = FFN // P      # 16  F-chunks (output partition tiles of stage A)
HC_N = HIDDEN // P   # 8   H-chunks (output partition tiles of stage B)
FCP_N = FC_N // 2    # 8   F-chunk pairs (one weight DMA each)
HCP_N = HC_N // 2    # 4   H-chunk pairs

BF16 = ml_dtypes.bfloat16

_compiled = {}  # cap -> compiled Bacc module


def _build(cap: int):
    f32 = mybir.dt.float32
    bf16 = mybir.dt.bfloat16
    tchunks = [(t0, min(512, cap - t0)) for t0 in range(0, cap, 512)]

    nc = bacc.Bacc("TRN2", debug=False, enable_asserts=False,
                   num_devices=N_EXPERTS)
    xT_d = nc.dram_tensor("xT", [P, KO_H, cap], bf16, kind="ExternalInput")
    wu_d = nc.dram_tensor("wu", [FCP_N, P, 2, KO_H, P], bf16,
                          kind="ExternalInput")
    wg_d = nc.dram_tensor("wg", [FCP_N, P, 2, KO_H, P], bf16,
                          kind="ExternalInput")
    wd_d = nc.dram_tensor("wd", [HCP_N, P, 2, KO_F, P], bf16,
                          kind="ExternalInput")
    yT_d = nc.dram_tensor("yT", [HC_N, P, cap], bf16, kind="ExternalOutput")

    with tile.TileContext(nc) as tc:
        with (
            tc.tile_pool(name="persist", bufs=1) as persist,
            tc.tile_pool(name="wpool", bufs=3) as wpool,
            tc.tile_pool(name="spool", bufs=4) as spool,
            tc.tile_pool(name="psum", bufs=2, space="PSUM") as psum,
        ):
            # Junk warmup: occupy the PE from preamble-end until the first
            # weights land (~2us) and start the HAM un-throttle window.
            # Alternate PSUM banks so consecutive junk matmuls don't
            # serialize on the bank WAW hazard.
            warm = persist.tile([P, 512], bf16, tag="warm")
            nc.gpsimd.memset(warm[:], 0)
            pw = [psum.tile([P, 512], f32, tag="pwarm", name="pwarm")
                  for _ in range(2)]
            # 9 x 427ns (cold) bridges the PE from preamble-end (~7.9us)
            # to first-weights-landed (~11.7us) with no idle gap, so the
            # HAM un-throttle fires before the first real matmul.
            for i in range(9):
                nc.tensor.matmul(pw[i % 2], warm[:, :P], warm, start=True,
                                 stop=True)

            xT = persist.tile([P, KO_H, cap], bf16, tag="xT")
            ht = persist.tile([P, KO_F, cap], bf16, tag="ht")

            # Stage A: h^T[fc] = silu(up^T) * gate^T, per 128-wide F-chunk.
            # All input DMAs go on ONE queue in exact demand order (two
            # concurrent HWDGE queues starve each other on the shared DMA
            # engines), and x^T is quartered so the first accumulation
            # chain starts as soon as its first slices land.
            for fcp in range(FCP_N):
                wu_t = wpool.tile([P, 2, KO_H, P], bf16, tag="wu")
                wg_t = wpool.tile([P, 2, KO_H, P], bf16, tag="wg")
                if fcp == 0:
                    nc.sync.dma_start(wu_t[:, 0], wu_d.ap()[0][:, 0])
                    for q in range(4):
                        nc.sync.dma_start(xT[:, 2 * q:2 * q + 2],
                                          xT_d.ap()[:, 2 * q:2 * q + 2])
                    nc.sync.dma_start(wg_t[:, 0], wg_d.ap()[0][:, 0])
                    nc.sync.dma_start(wu_t[:, 1], wu_d.ap()[0][:, 1])
                    nc.sync.dma_start(wg_t[:, 1], wg_d.ap()[0][:, 1])
                else:
                    nc.sync.dma_start(wu_t[:], wu_d.ap()[fcp])
                    nc.sync.dma_start(wg_t[:], wg_d.ap()[fcp])
                for half in range(2):
                    fc = 2 * fcp + half
                    for t0, nt in tchunks:
                        pu = psum.tile([P, 512], f32, tag="pu",
                                       name="pu")[:, :nt]
                        pg = psum.tile([P, 512], f32, tag="pg",
                                       name="pg")[:, :nt]
                        for ko in range(KO_H):
                            nc.tensor.matmul(pu, wu_t[:, half, ko],
                                             xT[:, ko, t0:t0 + nt],
                                             start=(ko == 0),
                                             stop=(ko == KO_H - 1))
                        for ko in range(KO_H):
                            nc.tensor.matmul(pg, wg_t[:, half, ko],
                                             xT[:, ko, t0:t0 + nt],
                                             start=(ko == 0),
                                             stop=(ko == KO_H - 1))
                        su = spool.tile([P, 512], f32, tag="silu",
                                        name="su")[:, :nt]
                        nc.scalar.activation(
                            su, pu, mybir.ActivationFunctionType.Sigmoid)
                        nc.vector.tensor_mul(su, su, pu)
                        nc.vector.tensor_mul(ht[:, fc, t0:t0 + nt], su, pg)

            # Stage B: y^T[hc] = w_down^T @ h^T, per 128-wide H-chunk
            for hcp in range(HCP_N):
                wd_t = wpool.tile([P, 2, KO_F, P], bf16, tag="wd")
                nc.sync.dma_start(wd_t[:], wd_d.ap()[hcp])
                for half in range(2):
                    hc = 2 * hcp + half
                    # halve the final chunk so the PSUM-drain + DMA-out
                    # chain after the very last matmul is short (a smaller
                    # final chunk loses more to strided-DMA overhead than
                    # it saves in chain latency)
                    chunks = tchunks
                    if hc == HC_N - 1 and tchunks[-1][1] > 384:
                        t0l, ntl = tchunks[-1]
                        h1 = (ntl // 2 + 3) & ~3
                        chunks = tchunks[:-1] + [(t0l, h1),
                                                 (t0l + h1, ntl - h1)]
                    for t0, nt in chunks:
                        py = psum.tile([P, 512], f32, tag="py",
                                       name="py")[:, :nt]
                        for ko in range(KO_F):
                            nc.tensor.matmul(py, wd_t[:, half, ko],
                                             ht[:, ko, t0:t0 + nt],
                                             start=(ko == 0),
                                             stop=(ko == KO_F - 1))
                        yo = spool.tile([P, 512], bf16, tag="yo",
                                        name="yo")[:, :nt]
                        nc.vector.tensor_copy(yo, py)
                        nc.sync.dma_start(yT_d.ap()[hc][:, t0:t0 + nt], yo)

    nc.compile()
    return nc


def kernel(x, weights, top_weights, top_experts, w_up, w_gate, w_down):
    x = np.asarray(x, dtype=np.float32)
    tw = np.asarray(top_weights, dtype=np.float32)
    te = np.asarray(top_experts).astype(np.int64)
    w_up = np.asarray(w_up, dtype=np.float32)
    w_gate = np.asarray(w_gate, dtype=np.float32)
    w_down = np.asarray(w_down, dtype=np.float32)

    B, S, H = x.shape
    T = B * S
    xf = x.reshape(T, H)

    # --- host routing ---
    idxs, combine = [], []
    for e in range(N_EXPERTS):
        sel = te == e                       # [T, K]
        mask = sel.any(axis=1)
        idx = np.nonzero(mask)[0]
        w_tok = (tw * sel).sum(axis=1)      # [T]
        idxs.append(idx)
        combine.append(w_tok[idx].astype(np.float32))
    max_n = max(len(i) for i in idxs)
    cap = max(-(-max_n // 8) * 8, P)  # PSUM free dim <= 512 f32 per bank

    # --- per-core inputs ---
    in_maps = []
    for e in range(N_EXPERTS):
        idx = idxs[e]
        xg = np.zeros((cap, H), np.float32)
        xg[: len(idx)] = xf[idx]
        # xT[p, ko, t] = xg[t, ko*128+p]
        xT = np.ascontiguousarray(
            xg.T.reshape(KO_H, P, cap).transpose(1, 0, 2)).astype(BF16)
        # [fcp, p, j, ko, q] = w_up[ko*128+p, (2*fcp+j)*128+q]
        wu = np.ascontiguousarray(
            w_up[e].reshape(KO_H, P, FCP_N, 2, P)
            .transpose(2, 1, 3, 0, 4)).astype(BF16)
        wg = np.ascontiguousarray(
            w_gate[e].reshape(KO_H, P, FCP_N, 2, P)
            .transpose(2, 1, 3, 0, 4)).astype(BF16)
        wd = np.ascontiguousarray(
            w_down[e].reshape(KO_F, P, HCP_N, 2, P)
            .transpose(2, 1, 3, 0, 4)).astype(BF16)
        in_maps.append({"xT": xT, "wu": wu, "wg": wg, "wd": wd})

    # --- compile (cached) + run ---
    if cap not in _compiled:
        _compiled[cap] = _build(cap)
    nc = _compiled[cap]
    res = bass_utils.run_bass_kernel_spmd(
        nc, in_maps, core_ids=list(range(N_EXPERTS)))

    # --- combine on host ---
    out = np.zeros((T, H), np.float32)
    for e in range(N_EXPERTS):
        idx = idxs[e]
        yT = res.results[e]["yT"].astype(np.float32).reshape(H, cap)
        out[idx] += yT[:, : len(idx)].T * combine[e][:, None]
    return out.reshape(B, S, H)
